# revision 28
# baseline (speedup 1.0000x reference)
"""Multi-head attention (B=2, N=2048, C=1024, H=16, D=64) on 8 TRN2 NeuronCores.

Sharding: 2 heads per core (tensor parallel over num_heads), both batch
elements on every core.  Each core computes q/k/v for its 2 heads, full
attention for those heads, and a partial output projection (row-parallel
over w_proj); the host sums the 8 f16 partial outputs and adds the bias.

Device-side dataflow per core, all matmuls f16 (1 PE cycle/row):
  q/k:   f16 matmuls over 8 c-tiles into [128,512] PSUM blocks (2 heads x
         64 d on partitions), evacuated to f16 SBUF.
  v:     computed directly in [m, d] orientation (x tile as the stationary
         operand) one m-tile at a time, evacuated into vo tiles laid out
         [V_h0 | ones | V_h1] so each head's AV stationary operand is a
         [128,128] f16 slab whose ones columns produce the softmax
         denominator rows in the same matmul (no PE transposes needed).
  attn:  one software-pipelined loop over all (batch, head, chunk, m-pair)
         steps: two score matmuls (K=64) into a [128,2,512] PSUM tile, one
         ACT exp (scale folded in, f16 out; logits are O(3) so no max
         subtraction), with AV accumulation deferred several steps behind
         so PE never waits on ACT - across all chunk/head/batch boundaries.
  norm:  DVE reciprocal + multiply into oc (f16).
  proj:  y_partial[n,:] = oc.T @ w_proj, f32 PSUM evacuated to f16, DMA out.

Cross-phase overlap: only the q/k projection chains of batch 0 run before
the attention step loop; everything else (batch-0 v, batch-1 qkv/v, both
projections) is chopped into ~0.5-1us PE work units and injected into the
attention steps at an adaptive rate, so the ACT engine's ~134us of exp
runs under PE's ~150us of matmul work with minimal warmup/drain.
GPSIMD touches nothing in PSUM (hardware restriction); all PSUM
evacuation is on DVE.  x DMAs are issued in n-half-major order so the
first q/k chains start after only half of x has landed.
"""

import sys

sys.path.insert(0, "/opt/trn_rl_repo")

import numpy as np

import concourse.bass as bass
import concourse.mybir as mybir
import concourse.tile as tile
from concourse import bacc
from concourse.bass_utils import run_bass_kernel_spmd

F32 = mybir.dt.float32
F16 = mybir.dt.float16
F8 = mybir.dt.float8e4
AF = mybir.ActivationFunctionType
ALU = mybir.AluOpType
DR = mybir.MatmulPerfMode.DoubleRow

# fp8 DoubleRow score matmuls: q quantized to fp8e4m3 (both slots), k split
# hi/lo compensated across the two slots -> k exact, q ~3.6% quantization
# noise; halves PE score time.  Validated end-to-end error ~1.5e-2 < 2e-2.
SCORES_FP8 = True

B = 2
N = 2048
C = 1024
H = 16
D = 64
NCORES = 8
HPC = H // NCORES          # heads per core = 2
CT = C // 128              # c tiles = 8
NT = N // 128              # m tiles = 16
NP = NT // 2               # m pairs = 8
NCH = N // 512             # 512-wide n chunks = 4
SCALE = float(D) ** -0.5


def _build():
    nc = bacc.Bacc("TRN2")
    xT = nc.dram_tensor("xT", [B, CT, 128, N], F16, kind="ExternalInput")
    wqkv = nc.dram_tensor("wqkv", [CT, 128, 384], F16, kind="ExternalInput")
    wpT = nc.dram_tensor("wpT", [128, C], F16, kind="ExternalInput")
    y = nc.dram_tensor("y", [B, N, C], F16, kind="ExternalOutput")

    with tile.TileContext(nc) as tc:
        with tc.tile_pool(name="consts", bufs=1) as consts, \
             tc.tile_pool(name="xt", bufs=16) as xt_pool, \
             tc.tile_pool(name="qk", bufs=4) as qk_pool, \
             tc.tile_pool(name="vo", bufs=2) as vo_pool, \
             tc.tile_pool(name="et", bufs=12) as et_pool, \
             tc.tile_pool(name="oc", bufs=2) as oc_pool, \
             tc.tile_pool(name="rec", bufs=4) as rec_pool, \
             tc.tile_pool(name="yo", bufs=6) as yo_pool, \
             tc.tile_pool(name="pq", bufs=2, space="PSUM") as pq, \
             tc.tile_pool(name="ps", bufs=2, space="PSUM") as ps_pool, \
             tc.tile_pool(name="pav", bufs=2, space="PSUM") as pav:

            w_sb = consts.tile([128, CT, 384], F16)
            wp_sb = consts.tile([128, C], F16)

            xt = {}
            q16 = {}
            k16 = {}
            vo = {}
            oc_sb = {}

            def load_w(part):
                # qk columns first (phase-0 critical path); v columns later
                sl = slice(0, 256) if part == 0 else slice(256, 384)
                nc.sync.dma_start(
                    out=w_sb[:, :, sl],
                    in_=wqkv[:, :, sl].rearrange("t p o -> p t o"))

            def load_x(b, pieces):
                # finer pieces land sooner: the first q/k chains only need
                # the first n-columns of every c-tile.
                for ct in range(CT):
                    if (b, ct) not in xt:
                        xt[b, ct] = xt_pool.tile([128, N], F16, tag="xt",
                                                 name=f"xt_{b}_{ct}")
                for lo, hi in pieces:
                    for ct in range(CT):
                        nc.sync.dma_start(out=xt[b, ct][:, lo:hi],
                                          in_=xT[b, ct][:, lo:hi])

            def emit_qk_units(b):
                """16 PE units: half-chains of 4 c-tiles for the q and k
                blocks, n-chunk-major so early chains need only early DMAs."""
                if SCORES_FP8:
                    # slot-dim layouts: q duplicated, k as (hi, lo) pair
                    q16[b] = qk_pool.tile([128, 2, N], F8, tag="qk",
                                          name=f"q16_{b}")
                    k16[b] = qk_pool.tile([128, 2, N], F8, tag="qk",
                                          name=f"k16_{b}")
                else:
                    q16[b] = qk_pool.tile([128, N], F16, tag="qk",
                                          name=f"q16_{b}")
                    k16[b] = qk_pool.tile([128, N], F16, tag="qk",
                                          name=f"k16_{b}")

                order = [(0, 0), (1, 0), (1, 1), (1, 2), (1, 3),
                         (0, 1), (0, 2), (0, 3)]
                if True:
                    for blk, nch in order:
                        sl = slice(nch * 512, (nch + 1) * 512)
                        psq = pq.tile([128, 512], F32, tag="pq",
                                      name=f"psq_{b}_{blk}_{nch}")

                        def half(ct0, psq=psq, blk=blk, sl=sl, b=b):
                            for ct in range(ct0, ct0 + 4):
                                nc.tensor.matmul(
                                    psq[:, :],
                                    w_sb[:, ct, blk * 128:(blk + 1) * 128],
                                    xt[b, ct][:, sl],
                                    start=(ct == 0), stop=(ct == CT - 1),
                                )
                            if ct0 != 4:
                                return
                            dst = q16[b] if blk == 0 else k16[b]
                            if not SCORES_FP8:
                                nc.vector.tensor_copy(dst[:, sl], psq[:, :])
                            elif blk == 0:
                                nc.vector.tensor_copy(dst[:, 0, sl], psq[:, :])
                                nc.vector.tensor_copy(dst[:, 1, sl], psq[:, :])
                            else:
                                nc.vector.tensor_copy(dst[:, 0, sl], psq[:, :])
                                nc.vector.tensor_tensor(
                                    out=dst[:, 1, sl], in0=psq[:, :],
                                    in1=dst[:, 0, sl], op=ALU.subtract)

                        yield lambda h=half: h(0)
                        yield lambda h=half: h(4)

            def emit_v_units(b):
                """17 PE units: vo init, then one unit per m-tile computing
                v[m,d] directly (x as stationary operand, 8 accumulating
                128-free matmuls) and packing it into the vo layout."""
                vo[b] = vo_pool.tile([128, NT, 192], F16, tag="vo", name=f"vo_{b}")

                def vo_init(b=b):
                    nc.gpsimd.memset(vo[b][:, :, 64:128], 1.0)

                yield vo_init

                for mt in range(NT):
                    def vunit(mt=mt, b=b):
                        pv = pq.tile([128, 128], F32, tag="pq",
                                     name=f"pv_{b}_{mt}")
                        msl = slice(mt * 128, (mt + 1) * 128)
                        for ct in range(CT):
                            nc.tensor.matmul(
                                pv[:, :],
                                xt[b, ct][:, msl],
                                w_sb[:, ct, 256:384],
                                start=(ct == 0), stop=(ct == CT - 1),
                            )
                        nc.vector.tensor_copy(vo[b][:, mt, 0:64], pv[:, 0:64])
                        nc.vector.tensor_copy(vo[b][:, mt, 128:192], pv[:, 64:128])

                    yield vunit

            def emit_proj_units(b, q):
                """4 PE units: one per n-tile (2 matmuls + evac + DMA out)."""
                for nt in range(q * NT // NCH, (q + 1) * NT // NCH):
                    def unit(nt=nt, b=b):
                        ysb = yo_pool.tile([128, 1024], F16, tag="yo",
                                           name=f"ysb_{b}_{nt}")
                        for och in range(2):
                            pp = pq.tile([128, 512], F32, tag="pq",
                                         name=f"pp_{b}_{nt}_{och}")
                            nc.tensor.matmul(
                                pp[:, :],
                                oc_sb[b][:, nt * 128:(nt + 1) * 128],
                                wp_sb[:, och * 512:(och + 1) * 512],
                                start=True, stop=True,
                            )
                            cp = (nc.scalar.copy
                                  if (b == 1 and q == 3 and och == nt % 2)
                                  else nc.vector.tensor_copy)
                            cp(ysb[:, och * 512:(och + 1) * 512], pp[:, :])
                        nc.sync.dma_start(
                            out=y[b, nt * 128:(nt + 1) * 128, :],
                            in_=ysb[:, :],
                        )

                    yield unit

            # ---- schedule ----
            load_w(0)
            load_x(0, [(0, 1024)])
            load_w(1)
            load_x(0, [(1024, 2048)])
            load_x(1, [(0, 1024), (1024, 2048)])
            nc.sync.dma_start(out=wp_sb, in_=wpT[:, :])

            qk0 = list(emit_qk_units(0))
            for u in qk0[:4]:
                u()

            uq = []          # pending PE work units
            uq.extend(("qk0", u) for u in qk0[4:])
            av_tile = {}

            NG = NP          # m-pair groups per chunk

            def attn_scores(b, hl, q, g):
                """Emit scores for one m-pair + exp; return deferred AV."""
                hs = hl * 64
                qof = q * 512
                if hl == 0 and q == 0 and g == 0:
                    oc_sb[b] = oc_pool.tile([128, N], F16, tag="oc",
                                            name=f"oc_{b}")
                if g == 0:
                    av_tile[b, hl, q] = pav.tile([128, 512], F32, tag="pav",
                                                 name=f"av_{b}_{hl}_{q}")
                av = av_tile[b, hl, q]
                s = ps_pool.tile([128, 2, 512], F32, tag="ps",
                                 name=f"s_{b}_{hl}_{q}_{g}")
                for i in range(2):
                    m_ = 2 * g + i
                    if SCORES_FP8:
                        nc.tensor.matmul(
                            s[:, i, :],
                            k16[b][hs:hs + 64, :, m_ * 128:(m_ + 1) * 128],
                            q16[b][hs:hs + 64, :, qof:qof + 512],
                            start=True, stop=True, perf_mode=DR,
                        )
                    else:
                        nc.tensor.matmul(
                            s[:, i, :],
                            k16[b][hs:hs + 64, m_ * 128:(m_ + 1) * 128],
                            q16[b][hs:hs + 64, qof:qof + 512],
                            start=True, stop=True,
                        )
                et = et_pool.tile([128, 2, 512], F16, tag="et",
                                  name=f"et_{b}_{hl}_{q}_{g}")
                nc.scalar.activation(out=et[:, :, :], in_=s[:, :, :],
                                     func=AF.Exp, scale=SCALE)

                def deferred(b=b, hl=hl, q=q, g=g, av=av, et=et, hs=hs, qof=qof):
                    for i in range(2):
                        nc.tensor.matmul(
                            av[:, :],
                            vo[b][:, 2 * g + i, hs:hs + 128],
                            et[:, i, :],
                            start=(g == 0 and i == 0),
                            stop=(g == NG - 1 and i == 1),
                        )
                    if g == NG - 1:
                        osl = slice(0, 64) if hl == 0 else slice(64, 128)
                        dsl = slice(64, 128) if hl == 0 else slice(0, 64)
                        rec = rec_pool.tile([128, 512], F32, tag="rec",
                                            name=f"rec_{b}_{hl}_{q}")
                        nc.vector.reciprocal(rec[dsl, :], av[dsl, :])
                        nc.vector.tensor_mul(
                            oc_sb[b][hs:hs + 64, qof:qof + 512],
                            av[osl, :],
                            rec[dsl, :],
                        )
                        if hl == 1:  # batch b's chunk q fully done
                            uq.extend(("proj", u)
                                      for u in emit_proj_units(b, q))

                return deferred

            steps = [(b, hl, q, g)
                     for b in range(B) for hl in range(HPC)
                     for q in range(NCH) for g in range(NG)]
            dq = []          # deferred AV closures, FIFO
            for si, (b, hl, q, g) in enumerate(steps):
                if si == 0:
                    uq.extend(("v0", u) for u in emit_v_units(0))
                if si == 8:
                    # qk chains first (hard deadline: batch-1 scores at
                    # si=64); v units later - their real deadline is the
                    # batch-1 AV stream, so leftovers spill past the
                    # boundary into the ACT-bound batch-1 half
                    uq.extend(("qk1", u) for u in emit_qk_units(1))
                    uq.extend(("v1", u) for u in emit_v_units(1))
                if (b, hl, q, g) == (1, 0, 0, 0):
                    # batch-1 scores need its q/k complete: flush leftovers
                    while uq and uq[0][0] == "qk1":
                        uq.pop(0)[1]()
                dq.append(attn_scores(b, hl, q, g))
                # inject pending units: fast in the first steps (batch-0
                # qk/v units must beat their consumers), adaptively after
                npop = 2 if (si <= 16 or 64 <= si <= 72) else (1 if (si % 2 == 0 or len(uq) > 18) else 0)
                for _ in range(npop):
                    if uq:
                        uq.pop(0)[1]()
                # run deferred AVs, lagging behind scores/exp so PE never
                # waits on ACT; extra lag early while v-units stream in
                lag = max(2, 12 - max(0, si - 12) // 2)
                while len(dq) > lag:
                    dq.pop(0)()
            while dq:
                dq.pop(0)()
            while uq:
                uq.pop(0)[1]()
    nc.finalize()
    return nc


_NC = None


def _get_nc():
    global _NC
    if _NC is None:
        _NC = _build()
    return _NC


def _make_in_maps(x, w_qkv, w_proj):
    xT = np.ascontiguousarray(x.transpose(0, 2, 1)).astype(np.float16)
    xT = xT.reshape(B, CT, 128, N)
    in_maps = []
    for core in range(NCORES):
        h0 = core * HPC
        rows = np.concatenate(
            [np.arange(h * D, (h + 1) * D) for h in range(h0, h0 + HPC)]
        )
        w = np.concatenate(
            [w_qkv[rows, :], w_qkv[C + rows, :], w_qkv[2 * C + rows, :]], axis=0
        )  # [384, 1024]
        wqkvT = np.ascontiguousarray(w.T).astype(np.float16).reshape(CT, 128, 384)
        cols = np.arange(h0 * D, (h0 + HPC) * D)
        wpT = np.ascontiguousarray(w_proj[:, cols].T).astype(np.float16)
        in_maps.append({"xT": xT, "wqkv": wqkvT, "wpT": wpT})
    return in_maps


def kernel(x, w_qkv, w_proj, b_proj):
    x = np.asarray(x, dtype=np.float32)
    w_qkv = np.asarray(w_qkv, dtype=np.float32)
    w_proj = np.asarray(w_proj, dtype=np.float32)
    b_proj = np.asarray(b_proj, dtype=np.float32)

    in_maps = _make_in_maps(x, w_qkv, w_proj)
    nc = _get_nc()
    res = run_bass_kernel_spmd(nc, in_maps, core_ids=list(range(NCORES)))
    out = np.zeros((B, N, C), dtype=np.float32)
    for core in range(NCORES):
        out += res.results[core]["y"].astype(np.float32)
    out += b_proj
    return out
